# revision 39
# baseline (speedup 1.0000x reference)
"""RBF-kernel attention (nn_Attention_76081050682051) on 8 TRN2 NeuronCores.

Self-contained Bass/Tile kernel. `kernel(**inputs)` takes the FULL unsharded
inputs of reference.setup_inputs() and returns the FULL [4, 2048, 256] f32
output.

Sharding (B x tensor-parallel heads): core c -> batch b = c//2, heads
[4*(c%2), 4*(c%2)+4); pairwise AllReduce ([0,1],[2,3],[4,5],[6,7]) combines
the two half-head partial outputs of each batch after the W_o projection.

Device math (all numerics device-verified; ~5.1e-3 rel err vs 2e-2 gate):
  x ships as fp16 (N(0,1) data; 10-bit mantissa costs ~0.1% in scores).
  LayerNorm per-partition stats via bn_stats/bn_aggr; rsqrt via DVE
  reciprocal_approx_fast + one ACT Sqrt; normalize on GPSIMD; xnT built by
  PE transposes batched 4-per-psum-tile.  Weights are DMA'd straight into
  f32r tiles (host arrays are plain f32; walrus accepts DMA producers).
  Per head: K'T/Q'T = (folded W).T @ xnT in f32r, then quantized to fp8e4
  hi+lo pairs.  The QK^T scores matmul runs as 3 fp8 DoubleRow matmuls per
  [128,512] tile (hi*hi + hi*lo + lo*hi, contraction 256 per instr at 0.5
  cyc/row) - 1.33x the f32r rate at ~0.4% error.  V projections for heads
  1-3 are also 3-term fp8 DoubleRow (xn8 = fp8(8*xn) hi/lo converted during
  head 0's window, W8 = fp8(64*W) from the host, 1/512 descale in the vt
  drain); safe because V has no exp downstream (see FP8_V note).
  k2/q2 row sums: per-feature squares (ACT Square on the warmup path, Pool
  hi+lo reconstruction steady-state, bias matmuls deferred one prefetch
  slot) then [128,1]-output plain-fp32 ones-matmuls put k2/q2 directly on
  partitions - no DRAM roundtrip.
  scoresT[t,s] = exp(qk'[t,s] - k2'[t]/2) via one ACT exp per [128,512]
  tile (per-partition bias); exp(-q2'[s]/2) is applied after W_o as a
  per-partition scale.  outT = V.T @ scoresT accumulates over t in f32r
  (fp8 needs >=8 mantissa bits here and outT inherits the unnormalized
  scores' e^+-17 per-column range - both over budget).  W_o runs on outT
  column slices; partial outputs AllReduce within each batch pair at
  quarter granularity (last quarter split further to shrink the tail).
  Emission is software-pipelined across heads: LayerNorm interleaves with
  head-0 K projections, each s-block main pre-emits the next block's first
  two score tiles during its AV-only tail (the exp chain never gates the
  in-order PE stream at boundaries), and projection/V prefetch for head
  h+1 rides inside head h's mains.
"""
import sys
sys.path.insert(0, '/opt/trn_rl_repo')
import numpy as np
from concourse import bass, bacc, tile, mybir, masks
from concourse.bass_utils import run_bass_kernel_spmd

F32 = mybir.dt.float32
F32R = mybir.dt.float32r
F16 = mybir.dt.float16
FP8 = mybir.dt.float8e4
AF = mybir.ActivationFunctionType
OP = mybir.AluOpType
DR = mybir.MatmulPerfMode.DoubleRow

B, S, E, H = 4, 2048, 256, 8
HL = 4          # heads per core
EC = 2          # e chunks of 128
SB = 4          # s blocks of 512
ST = 16         # s/t tiles of 128
N_CORES = 8
EPS = 1e-5

NO_COLL = False
N_HEADS_BUILD = HL
# fp8 DoubleRow projections for heads 2-3: ~3.7us faster but adds
# ~7.6e-3 relative error.  The ~1.2e-3 per-element error of the 3-term
# hi/lo product (dropped lo*lo + both lo-requantizations) amplifies by
# sqrt(E)=16 through the qk contraction into ~0.5% score error per
# affected head, plus correlated k2/q2 bias errors -- fundamental to the
# split, and a 4th term would erase the PE savings.  Disabled to keep a
# 4x margin to the 2e-2 correctness gate.
FP8_PROJ = False
# fp8 DoubleRow V projections for heads 1-3 ARE safe: V has no exp
# downstream, so its ~1.2e-3 per-element hi/lo error reaches the output
# at ~1.2e-3 (no sqrt(E) amplification), and the vt drain stays one DVE
# op (descale instead of copy).  Head 0's V runs before the xn8
# conversion window, so it stays f32r.
FP8_V = True


def build_kernel(R=1, debug=False):
    nc = bacc.Bacc("TRN2", target_bir_lowering=False, debug=False,
                   num_devices=N_CORES)

    x_ext = nc.declare_dram_parameter("x", [S, E], F16, isOutput=False)
    w_ext = {}
    for wname in ("wq", "wk", "wv", "wo"):
        w_ext[wname] = nc.declare_dram_parameter(wname, [HL, 128, EC * E],
                                                 F32R, isOutput=False)
    for wname in ("wq", "wk", "wv"):
        for part in ("h", "l"):
            w_ext[wname + "8" + part] = nc.declare_dram_parameter(
                wname + "8" + part, [HL, 128, EC, E], FP8, isOutput=False)
    out_ext = nc.declare_dram_parameter("out", [S, E], F32, isOutput=True)

    with tile.TileContext(nc) as tc:
        with tc.tile_pool(name="sb", bufs=1) as sb, \
             tc.tile_pool(name="sbt", bufs=1) as sbt, \
             tc.tile_pool(name="ps", bufs=1, space="PSUM") as ps, \
             tc.tile_pool(name="dram", bufs=1, space="DRAM") as dram:

            ones32 = sb.tile([128, 1], F32, name="ones32")
            nc.any.memset(ones32[:], 1.0)
            ident128 = sb.tile([128, 128], F32, name="ident128")
            masks.make_identity(nc, ident128[:])

            xu_tiles = []
            for sbk in range(SB):
                xu = sbt.tile([128, 4 * E], F16, name="xu", tag="xu", bufs=4)
                for hv in range(2):
                    nc.sync.dma_start(
                        xu[:, 2 * hv * E:2 * (hv + 1) * E]
                        .rearrange("p (t e) -> p t e", t=2),
                        x_ext[sbk * 512 + hv * 256:sbk * 512 + (hv + 1) * 256, :]
                        .rearrange("(t p) e -> p t e", p=128))
                xu_tiles.append(xu)

            pools = dict(sb=sb, sbt=sbt, ps=ps, dram=dram)
            _build_body(nc, tc, pools, xu_tiles, w_ext, ones32, ident128,
                        out_ext)

    nc.compile()
    return nc


def _build_body(nc, tc, pools, xu_tiles, w_ext, ones32, ident128, out_ext):
    sb, sbt, ps, dram = pools['sb'], pools['sbt'], pools['ps'], pools['dram']

    def big_ps(tag="pp", bufs=2):
        return ps.tile([128, 512], F32, name=tag, tag=tag, bufs=bufs)

    def sm_ps():
        # shared small-psum ring: pstk/pstq/pv/wops all [128, 256]
        return ps.tile([128, 256], F32, name="sm", tag="sm", bufs=2)

    # ============ LayerNorm pieces (called per s-block) ============
    xn = {}
    for ec in range(EC):
        for sbk in range(SB):
            xn[ec, sbk] = sb.tile([128, 512], F32R, name=f"xn_{ec}_{sbk}")
    xn8 = {}
    for part in ("h", "l"):
        for sbk in range(SB):
            xn8[part, sbk] = sb.tile([128, 2, 512], FP8,
                                     name=f"xn8{part}_{sbk}")

    def emit_xn8(sbk):
        """fp8 hi/lo of 8*xnT for one s-block (feeds head>=2 projections);
        the x8 keeps the lo residuals out of fp8e4's subnormal range.
        Emitted during head 0's prefetch slots where ACT/DVE have slack."""
        for ec in range(EC):
            nc.scalar.activation(xn8["h", sbk][:, ec, :], xn[ec, sbk][:],
                                 AF.Identity, scale=8.0)
            nc.vector.scalar_tensor_tensor(xn8["l", sbk][:, ec, :],
                                           xn[ec, sbk][:], 8.0,
                                           xn8["h", sbk][:, ec, :],
                                           OP.mult, OP.subtract)

    def emit_ln(sbk):
        xu = xu_tiles[sbk]
        st6 = sbt.tile([128, 4, 6], F32, name="st6", tag="st6", bufs=2)
        mv = sbt.tile([128, 4, 2], F32, name="mv", tag="mv", bufs=2)
        vb = sbt.tile([128, 4], F32, name="vb", tag="vb", bufs=2)
        inv4 = sbt.tile([128, 4], F32, name="inv4", tag="inv4", bufs=2)
        for j in range(4):
            nc.vector.bn_stats(st6[:, j], xu[:, j * E:(j + 1) * E])
            nc.vector.bn_aggr(mv[:, j], st6[:, j])
        rcp = sbt.tile([128, 4], F32, name="rcp", tag="rcp", bufs=2)
        nc.vector.tensor_scalar_add(vb[:], mv[:, :, 1], EPS)
        with nc.allow_low_precision("~18-bit reciprocal + table sqrt is"
                                    " plenty for a LN scale"):
            nc.vector.reciprocal_approx_fast(rcp[:], vb[:])
        nc.scalar.activation(inv4[:], rcp[:], AF.Sqrt, scale=1.0)
        xnus = []
        for j in range(4):
            xnu = sbt.tile([128, E], F32, name="xnu", tag="xnu", bufs=6)
            nc.gpsimd.tensor_scalar(xnu[:], xu[:, j * E:(j + 1) * E],
                                    mv[:, j, 0:1], inv4[:, j:j + 1],
                                    OP.subtract, OP.mult)
            xnus.append(xnu)
        for ec in range(EC):
            pt2 = big_ps()
            for j in range(4):
                nc.tensor.transpose(pt2[:, j * 128:(j + 1) * 128],
                                    xnus[j][:, ec * 128:(ec + 1) * 128],
                                    ident128[:])
            if ec == 0:
                nc.scalar.copy(xn[ec, sbk][:], pt2[:])
            else:
                nc.vector.tensor_copy(xn[ec, sbk][:], pt2[:])

    def xn_col(ec, st):
        sbk, j = divmod(st, 4)
        return xn[ec, sbk][:, j * 128:(j + 1) * 128]

    SL = [slice(i * 512, (i + 1) * 512) for i in range(SB)]

    # ============ per-head state ============
    acc = sb.tile([128, ST * E], F32, name="acc")

    bounce_in = [dram.tile([S // 2, E], F32, name=f"bounce_in{i}",
                           tag=f"bin{i}", bufs=1) for i in range(2)]
    bounce_view = [b.rearrange("(t p) e -> p t e", p=128) for b in bounce_in]

    st_h = {}

    def new_head_state(h):
        w = {}
        names = ["wo"]
        if h < 2 or not FP8_PROJ:
            names += ["wk", "wq"]
            if h == 0 or not FP8_V:
                names.append("wv")
        for wname in names:
            wr = sbt.tile([128, EC * E], F32R, name=f"w_{wname}",
                          tag=f"w_{wname}", bufs=2)
            nc.sync.dma_start(wr[:], w_ext[wname][h])
            w[wname] = wr
        w8names = []
        if FP8_PROJ and h >= 2:
            w8names += ["wk", "wq", "wv"]
        elif FP8_V and h >= 1:
            w8names.append("wv")
        for wname in w8names:
            for part in ("h", "l"):
                w8 = sbt.tile([128, EC, E], FP8, name=f"w8_{wname}{part}",
                              tag=f"w8_{wname}{part}", bufs=2)
                nc.sync.dma_start(w8[:], w_ext[wname + "8" + part][h])
                w[wname + "8" + part] = w8
        st_h[h] = dict(w=w, khi={}, klo={}, qhi={}, qlo={}, vt={}, outT={},
                       biasq={}, eq2q={}, sqk={}, sqq={})

    def emit_projA(h, sbk, which):
        """f32r projection of K^T or Q^T for one s-block + fp8 hi/lo
        quantization + per-feature squares (head-0 K on ACT for the warmup
        critical path; otherwise reconstructed from hi+lo on the idle Pool
        engine, which cannot read PSUM)."""
        s = st_h[h]
        wr = s['w'].get('wk' if which == 'k' else 'wq')
        hi = sbt.tile([128, 2, 512], FP8, name=which + "hi", tag=which + "hi",
                      bufs=8)
        lo = sbt.tile([128, 2, 512], FP8, name=which + "lo", tag=which + "lo",
                      bufs=8)
        sqs = []
        for ft in range(EC):
            pp = big_ps()
            if h < 2 or not FP8_PROJ:
                for ec in range(EC):
                    o = ec * E + ft * 128
                    nc.tensor.matmul(pp[:], wr[:, o:o + 128],
                                     xn[ec, sbk][:],
                                     start=(ec == 0), stop=(ec == EC - 1))
            else:
                wn = 'wk' if which == 'k' else 'wq'
                w8h = s['w'][wn + '8h']
                w8l = s['w'][wn + '8l']
                o = ft * 128
                xh, xl = xn8["h", sbk][:], xn8["l", sbk][:]
                nc.tensor.matmul(pp[:], w8h[:, :, o:o + 128], xh,
                                 start=True, stop=False, perf_mode=DR)
                nc.tensor.matmul(pp[:], w8h[:, :, o:o + 128], xl,
                                 start=False, stop=False, perf_mode=DR)
                nc.tensor.matmul(pp[:], w8l[:, :, o:o + 128], xh,
                                 start=False, stop=True, perf_mode=DR)
            if FP8_PROJ and h >= 2:
                # fp8-weight projection left pp scaled by 512
                nc.vector.tensor_scalar_mul(hi[:, ft, :], pp[:], 1.0 / 512.0)
                nc.vector.scalar_tensor_tensor(lo[:, ft, :], pp[:],
                                               1.0 / 512.0, hi[:, ft, :],
                                               OP.mult, OP.subtract)
            else:
                if h == 0 and (which == 'k' or sbk <= 1):
                    nc.scalar.copy(hi[:, ft, :], pp[:])
                else:
                    nc.vector.tensor_copy(hi[:, ft, :], pp[:])
                nc.vector.tensor_tensor(lo[:, ft, :], pp[:], hi[:, ft, :],
                                        OP.subtract)
            sq = sbt.tile([128, 512], F32, name="sq", tag="sq", bufs=10)
            if h == 0 and which == 'k':
                nc.scalar.activation(sq[:], pp[:], AF.Square, scale=1.0)
            else:
                tsum = sbt.tile([128, 512], F32, name="tsum", tag="tsum",
                                bufs=2)
                nc.gpsimd.tensor_tensor(tsum[:], hi[:, ft, :], lo[:, ft, :],
                                        OP.add)
                nc.gpsimd.tensor_tensor(sq[:], tsum[:], tsum[:], OP.mult)
            sqs.append(sq)
        if which == 'k':
            s['khi'][sbk], s['klo'][sbk] = hi, lo
            s['sqk'][sbk] = sqs
        else:
            s['qhi'][sbk], s['qlo'][sbk] = hi, lo
            s['sqq'][sbk] = sqs

    def emit_bias(h, sbk, which):
        """k2/q2 per-partition columns via [128,1] fp32 ones-matmuls.
        Emitted a prefetch slot after emit_projA so the PE stream never
        waits on the Pool square chain."""
        s = st_h[h]
        sqs = (s['sqk'] if which == 'k' else s['sqq']).pop(sbk)
        pst = sm_ps()
        for j in range(4):
            for ft in range(EC):
                nc.tensor.matmul(pst[:, j:j + 1],
                                 sqs[ft][:, j * 128:(j + 1) * 128],
                                 ones32[:], start=(ft == 0),
                                 stop=(ft == EC - 1))
        if which == 'k':
            bq = sbt.tile([128, 4], F32, name="biasq", tag="biasq", bufs=8)
            nc.vector.tensor_scalar_mul(bq[:], pst[:, 0:4], -0.5)
            s['biasq'][sbk] = bq
        else:
            eq = sbt.tile([128, 4], F32, name="eq2q", tag="eq2q", bufs=8)
            nc.scalar.activation(eq[:], pst[:, 0:4], AF.Exp, scale=-0.5)
            s['eq2q'][sbk] = eq

    def emit_proj(h, sbk, which):
        emit_projA(h, sbk, which)
        emit_bias(h, sbk, which)

    def emit_v(h, sbk):
        s = st_h[h]
        wv = s['w'].get('wv')
        for st in range(sbk * 4, sbk * 4 + 4):
            xsb, xj = divmod(st, 4)
            csl = slice(xj * 128, (xj + 1) * 128)
            pv = sm_ps()
            if not ((FP8_V and h >= 1) or (FP8_PROJ and h >= 2)):
                for ec in range(EC):
                    nc.tensor.matmul(pv[:], xn_col(ec, st),
                                     wv[:, ec * E:(ec + 1) * E],
                                     start=(ec == 0), stop=(ec == EC - 1))
            else:
                wvh, wvl = s['w']['wv8h'], s['w']['wv8l']
                nc.tensor.matmul(pv[:], xn8["h", xsb][:, :, csl], wvh[:],
                                 start=True, stop=False, perf_mode=DR)
                nc.tensor.matmul(pv[:], xn8["h", xsb][:, :, csl], wvl[:],
                                 start=False, stop=False, perf_mode=DR)
                nc.tensor.matmul(pv[:], xn8["l", xsb][:, :, csl], wvh[:],
                                 start=False, stop=True, perf_mode=DR)
            v = sbt.tile([128, E], F32R, name="vt", tag="vt", bufs=24)
            if (FP8_V and h >= 1) or (FP8_PROJ and h >= 2):
                nc.vector.tensor_scalar_mul(v[:], pv[:], 1.0 / 512.0)
            else:
                nc.vector.tensor_copy(v[:], pv[:])
            s['vt'][st] = v

    sc_pre = {}

    def emit_score_tile(h, sbk, tt):
        """Scores + exp for one [128t, 512s] tile (3 fp8 DoubleRow matmuls
        + one biased ACT exp)."""
        s = st_h[h]
        tb, tj = divmod(tt, 4)
        csl = slice(tj * 128, (tj + 1) * 128)
        kh = s['khi'][tb][:, :, csl]
        kl = s['klo'][tb][:, :, csl]
        qh, ql = s['qhi'][sbk][:], s['qlo'][sbk][:]
        stps = big_ps(tag="stps", bufs=2)
        nc.tensor.matmul(stps[:], kh, qh, start=True, stop=False,
                         perf_mode=DR)
        nc.tensor.matmul(stps[:], kh, ql, start=False, stop=False,
                         perf_mode=DR)
        nc.tensor.matmul(stps[:], kl, qh, start=False, stop=True,
                         perf_mode=DR)
        sc = sbt.tile([128, 512], F32R, name="sc", tag="sc", bufs=12)
        nc.scalar.activation(sc[:], stps[:], AF.Exp,
                             bias=s['biasq'][tb][:, tj:tj + 1], scale=1.0)
        return sc

    def emit_main(h, sbk, warm_next=None):
        """Main loop for one s-block.  warm_next=(h', sbk') pre-emits that
        block's first SKEW score tiles during this block's AV-only tail so
        the next main never waits on the exp chain."""
        s = st_h[h]
        vt = s['vt']
        ops = [big_ps(tag="ov", bufs=2) for _ in range(EC)]
        sc_q = {}
        # head 0 s-block 0 runs more scores ahead of the AVs so the PE
        # stream is not blocked by the warmup DVE/ACT quantization backlog.
        SKEW = 8 if (h == 0 and sbk == 0) else 2
        for tt in range(ST + SKEW):
            if tt < ST:
                if (h, sbk, tt) in sc_pre:
                    sc_q[tt] = sc_pre.pop((h, sbk, tt))
                else:
                    sc_q[tt] = emit_score_tile(h, sbk, tt)
            elif warm_next is not None:
                wh, wsbk = warm_next
                wt = tt - ST
                if wt < 2:
                    sc_pre[(wh, wsbk, wt)] = emit_score_tile(wh, wsbk, wt)
            if tt >= SKEW:
                pv_tt = tt - SKEW
                sc_prev = sc_q.pop(pv_tt)
                for ft in range(EC):
                    nc.tensor.matmul(ops[ft][:],
                                     vt[pv_tt][:, ft * 128:(ft + 1) * 128],
                                     sc_prev[:],
                                     start=(pv_tt == 0), stop=(pv_tt == ST - 1))
        for ft in range(EC):
            o = sbt.tile([128, 512], F32R, name="outT", tag="outT", bufs=8)
            nc.vector.tensor_copy(o[:, 0:256], ops[ft][:, 0:256])
            nc.scalar.copy(o[:, 256:512], ops[ft][:, 256:512])
            s['outT'][ft, sbk] = o

    def emit_wo(h, sbk):
        s = st_h[h]
        wo = s['w']['wo']
        for st in range(sbk * 4, sbk * 4 + 4):
            j = st % 4
            wops = sm_ps()
            for ft in range(EC):
                nc.tensor.matmul(wops[:],
                                 s['outT'][ft, sbk][:, j * 128:(j + 1) * 128],
                                 wo[:, ft * E:(ft + 1) * E],
                                 start=(ft == 0), stop=(ft == EC - 1))
            asl = acc[:, st * E:(st + 1) * E]
            qb, qj = divmod(st, 4)
            eqcol = s['eq2q'][qb][:, qj:qj + 1]
            if h == 0:
                nc.vector.tensor_scalar(asl, wops[:], eqcol, None, OP.mult)
            else:
                nc.vector.scalar_tensor_tensor(asl, wops[:], eqcol,
                                               asl, OP.mult, OP.add)
        if h == N_HEADS_BUILD - 1:
            half, sth = divmod(sbk * 4, 8)
            tgt = bounce_view[half][:, sth:sth + 4, :]
            if sbk < SB - 1:
                nc.sync.dma_start(
                    tgt,
                    acc[:, sbk * 4 * E:(sbk + 1) * 4 * E]
                    .rearrange("p (t e) -> p t e", e=E))
            else:
                nc.sync.dma_start(
                    tgt[:, 0:2, :],
                    acc[:, sbk * 4 * E:(sbk * 4 + 2) * E]
                    .rearrange("p (t e) -> p t e", e=E))
                for ei in range(2, 4):
                    st0 = sbk * 4 + ei
                    nc.sync.dma_start(
                        tgt[:, ei:ei + 1, :],
                        acc[:, st0 * E:(st0 + 1) * E]
                        .rearrange("p (t e) -> p t e", e=E))

    # ============ emission schedule ============
    if N_HEADS_BUILD == 0:
        nc.any.memset(acc[:], 0.0)
    else:
        # LN interleaved with head-0 K projections: main(0,0) needs K/k2 of
        # all four s-blocks, so those quantization chains are the warmup
        # critical path (K squares on ACT there, bias inline).
        for sbk in range(SB):
            emit_ln(sbk)
            if sbk == 0:
                new_head_state(0)
            emit_proj(0, sbk, 'k')
        emit_projA(0, 0, 'q')
        emit_projA(0, 1, 'q')
        for sbk in range(SB):
            emit_v(0, sbk)
        emit_bias(0, 0, 'q')
        emit_bias(0, 1, 'q')

    for h in range(N_HEADS_BUILD):
        nxt = h + 1
        if nxt < N_HEADS_BUILD:
            new_head_state(nxt)
        for sbk in range(SB):
            if sbk < SB - 1:
                wn = (h, sbk + 1)
            elif nxt < N_HEADS_BUILD:
                wn = (nxt, 0)
            else:
                wn = None
            emit_main(h, sbk, warm_next=wn)
            if h == 0:
                # finish head 0's own pieces
                if sbk == 0:
                    emit_projA(0, 2, 'q')
                    emit_projA(0, 3, 'q')
                elif sbk == 1:
                    emit_bias(0, 2, 'q')
                elif sbk == 2:
                    emit_bias(0, 3, 'q')
            if (FP8_V or FP8_PROJ) and h == 0:
                emit_xn8(sbk)
            if nxt < N_HEADS_BUILD:
                if sbk == 0:
                    if h > 0:
                        emit_bias(h, 2, 'q')
                    emit_projA(nxt, 0, 'k')
                    emit_projA(nxt, 1, 'k')
                elif sbk == 1:
                    if h > 0:
                        emit_bias(h, 3, 'q')
                    emit_projA(nxt, 2, 'k')
                    emit_projA(nxt, 3, 'k')
                    emit_bias(nxt, 0, 'k')
                    emit_bias(nxt, 1, 'k')
                elif sbk == 2:
                    emit_bias(nxt, 2, 'k')
                    emit_bias(nxt, 3, 'k')
                    emit_projA(nxt, 0, 'q')
                    emit_projA(nxt, 1, 'q')
                    emit_v(nxt, 0)
                    emit_v(nxt, 1)
                else:
                    emit_bias(nxt, 0, 'q')
                    emit_bias(nxt, 1, 'q')
                    emit_projA(nxt, 2, 'q')
                    emit_projA(nxt, 3, 'q')
                    emit_v(nxt, 2)
                    emit_v(nxt, 3)
            else:
                if sbk == 0:
                    emit_bias(h, 2, 'q')
                elif sbk == 1:
                    emit_bias(h, 3, 'q')
            emit_wo(h, sbk)
        if h > 0:
            st_h.pop(h - 1, None)

    if N_HEADS_BUILD == 0:
        for half in range(2):
            nc.sync.dma_start(
                bounce_view[half][:, :, :],
                acc[:, half * 8 * E:(half + 1) * 8 * E]
                .rearrange("p (t e) -> p t e", e=E))

    # ==== AllReduce over batch pair + store (quarters; last one split) ====
    bos = [dram.tile([S // 2, E], F32, name=f"bounce_out{i}",
                     tag=f"bout{i}", bufs=1) for i in range(2)]
    chunks = [(0, 0, 512), (0, 512, 512), (1, 0, 512), (1, 512, 256),
              (1, 768, 128), (1, 896, 128)]
    for half, r0, rn in chunks:
        rsl = slice(r0, r0 + rn)
        o0 = half * (S // 2) + r0
        osl = out_ext[o0:o0 + rn, :]
        if NO_COLL:
            nc.sync.dma_start(osl, bounce_in[half][rsl, :])
        else:
            nc.gpsimd.collective_compute(
                "AllReduce", OP.add,
                replica_groups=[[0, 1], [2, 3], [4, 5], [6, 7]],
                ins=[bounce_in[half][rsl, :].opt()],
                outs=[bos[half][rsl, :].opt()],
            )
            nc.sync.dma_start(osl, bos[half][rsl, :])


# ================= host side =================

def prep_inputs(x, ln_scale, W_q, W_k, W_v, W_o, gamma):
    """Build per-core input maps."""
    x = np.asarray(x, np.float32)
    ln_scale = np.asarray(ln_scale, np.float32)
    W_q = np.asarray(W_q, np.float32)
    W_k = np.asarray(W_k, np.float32)
    W_v = np.asarray(W_v, np.float32)
    W_o = np.asarray(W_o, np.float32)
    gamma = np.asarray(gamma, np.float32).reshape(H)

    in_maps = []
    for c in range(N_CORES):
        b = c // 2
        h0 = HL * (c % 2)
        hs = list(range(h0, h0 + HL))
        g = gamma[hs]
        s2g = np.sqrt(2.0 * g).astype(np.float32)
        wq = (W_q[hs] * ln_scale[None, :, None] * s2g[:, None, None])
        wk = (W_k[hs] * ln_scale[None, :, None] * s2g[:, None, None])
        wv = (W_v[hs] * ln_scale[None, :, None])
        def _lay(w):   # [HL, E_in(=EC*128), E] -> [HL, 128, EC*E]
            return np.ascontiguousarray(
                w.reshape(HL, EC, 128, E).transpose(0, 2, 1, 3).reshape(HL, 128, EC * E))
        import ml_dtypes
        FP8NP = ml_dtypes.float8_e4m3

        def _split8(w):   # [HL, 128, EC*E] -> fp8 hi/lo [HL, 128, EC, E]
            # x64 lifts the ~0.03-rms folded weights out of fp8e4's
            # subnormal floor; the PSUM drains descale by 1/64.
            w4 = w.reshape(HL, 128, EC, E) * np.float32(64.0)
            hi = w4.astype(FP8NP)
            lo = (w4 - hi.astype(np.float32)).astype(FP8NP)
            return np.ascontiguousarray(hi), np.ascontiguousarray(lo)

        wq = _lay(wq)
        wk = _lay(wk)
        wv = _lay(wv)
        wo = _lay(np.stack([W_o[:, 256 * h:256 * (h + 1)].T.copy() for h in hs]))
        wq8h, wq8l = _split8(wq)
        wk8h, wk8l = _split8(wk)
        wv8h, wv8l = _split8(wv)
        in_maps.append({
            "x": np.ascontiguousarray(x[b]).astype(np.float16),
            "wq": np.ascontiguousarray(wq),
            "wk": np.ascontiguousarray(wk),
            "wv": np.ascontiguousarray(wv),
            "wo": np.ascontiguousarray(wo),
            "wq8h": wq8h, "wq8l": wq8l,
            "wk8h": wk8h, "wk8l": wk8l,
            "wv8h": wv8h, "wv8l": wv8l,
        })
    return in_maps


def assemble_output(results):
    out = np.empty((B, S, E), np.float32)
    for b in range(B):
        out[b] = results[2 * b]["out"]
    return out


_NC_CACHE = {}


def _get_nc():
    if 'nc' not in _NC_CACHE:
        _NC_CACHE['nc'] = build_kernel(R=1, debug=False)
    return _NC_CACHE['nc']


def kernel(x, e=None, p=None, ln_scale=None, W_q=None, W_k=None, W_v=None,
           W_o=None, gamma=None, **_unused):
    """Full-input entry point. e and p are unused by the reference network
    (use_ppe=False config); they are accepted and ignored."""
    in_maps = prep_inputs(x, ln_scale, W_q, W_k, W_v, W_o, gamma)
    nc = _get_nc()
    res = run_bass_kernel_spmd(nc, in_maps, core_ids=list(range(N_CORES)))
    return assemble_output(res.results)


# revision 42
# speedup vs baseline: 1.0007x; 1.0007x over previous
"""RBF-kernel attention (nn_Attention_76081050682051) on 8 TRN2 NeuronCores.

Self-contained Bass/Tile kernel. `kernel(**inputs)` takes the FULL unsharded
inputs of reference.setup_inputs() and returns the FULL [4, 2048, 256] f32
output.

Sharding (B x tensor-parallel heads): core c -> batch b = c//2, heads
[4*(c%2), 4*(c%2)+4); pairwise AllReduce ([0,1],[2,3],[4,5],[6,7]) combines
the two half-head partial outputs of each batch after the W_o projection.

Device math (all numerics device-verified; ~5.1e-3 rel err vs 2e-2 gate):
  x ships as fp16 (N(0,1) data; 10-bit mantissa costs ~0.1% in scores).
  LayerNorm per-partition stats via bn_stats/bn_aggr; rsqrt via DVE
  reciprocal_approx_fast + one ACT Sqrt; normalize on GPSIMD; xnT built by
  PE transposes batched 4-per-psum-tile.  Weights are DMA'd straight into
  f32r tiles (host arrays are plain f32; walrus accepts DMA producers).
  Per head: K'T/Q'T = (folded W).T @ xnT in f32r, then quantized to fp8e4
  hi+lo pairs.  The QK^T scores matmul runs as 3 fp8 DoubleRow matmuls per
  [128,512] tile (hi*hi + hi*lo + lo*hi, contraction 256 per instr at 0.5
  cyc/row) - 1.33x the f32r rate at ~0.4% error.  V projections for heads
  1-3 are also 3-term fp8 DoubleRow (xn8 = fp8(8*xn) hi/lo converted during
  head 0's window, W8 = fp8(64*W) from the host, 1/512 descale in the vt
  drain); safe because V has no exp downstream (see FP8_V note).
  k2/q2 row sums: per-feature squares (ACT Square on the warmup path, Pool
  hi+lo reconstruction steady-state, bias matmuls deferred one prefetch
  slot) then [128,1]-output plain-fp32 ones-matmuls put k2/q2 directly on
  partitions - no DRAM roundtrip.
  scoresT[t,s] = exp(qk'[t,s] - k2'[t]/2) via one ACT exp per [128,512]
  tile (per-partition bias); exp(-q2'[s]/2) is applied after W_o as a
  per-partition scale.  outT = V.T @ scoresT accumulates over t in f32r
  (fp8 needs >=8 mantissa bits here and outT inherits the unnormalized
  scores' e^+-17 per-column range - both over budget).  W_o runs on outT
  column slices; partial outputs AllReduce within each batch pair at
  quarter granularity (last quarter split further to shrink the tail).
  Emission is software-pipelined across heads: LayerNorm interleaves with
  head-0 K projections, each s-block main pre-emits the next block's first
  two score tiles during its AV-only tail (the exp chain never gates the
  in-order PE stream at boundaries), and projection/V prefetch for head
  h+1 rides inside head h's mains.
"""
import sys
sys.path.insert(0, '/opt/trn_rl_repo')
import numpy as np
from concourse import bass, bacc, tile, mybir, masks
from concourse.bass_utils import run_bass_kernel_spmd

F32 = mybir.dt.float32
F32R = mybir.dt.float32r
F16 = mybir.dt.float16
FP8 = mybir.dt.float8e4
AF = mybir.ActivationFunctionType
OP = mybir.AluOpType
DR = mybir.MatmulPerfMode.DoubleRow

B, S, E, H = 4, 2048, 256, 8
HL = 4          # heads per core
EC = 2          # e chunks of 128
SB = 4          # s blocks of 512
ST = 16         # s/t tiles of 128
N_CORES = 8
EPS = 1e-5

NO_COLL = False
N_HEADS_BUILD = HL
# fp8 DoubleRow projections for heads 2-3: ~3.7us faster but adds
# ~7.6e-3 relative error.  The ~1.2e-3 per-element error of the 3-term
# hi/lo product (dropped lo*lo + both lo-requantizations) amplifies by
# sqrt(E)=16 through the qk contraction into ~0.5% score error per
# affected head, plus correlated k2/q2 bias errors -- fundamental to the
# split, and a 4th term would erase the PE savings.  Disabled to keep a
# 4x margin to the 2e-2 correctness gate.
FP8_PROJ = False
# fp8 DoubleRow V projections for heads 1-3 ARE safe: V has no exp
# downstream, so its ~1.2e-3 per-element hi/lo error reaches the output
# at ~1.2e-3 (no sqrt(E) amplification), and the vt drain stays one DVE
# op (descale instead of copy).  Head 0's V runs before the xn8
# conversion window, so it stays f32r.
FP8_V = True


def build_kernel(R=1, debug=False):
    nc = bacc.Bacc("TRN2", target_bir_lowering=False, debug=False,
                   num_devices=N_CORES)

    x_ext = nc.declare_dram_parameter("x", [S, E], F16, isOutput=False)
    w_ext = {}
    for wname in ("wq", "wk", "wv", "wo"):
        w_ext[wname] = nc.declare_dram_parameter(wname, [HL, 128, EC * E],
                                                 F32R, isOutput=False)
    for wname in ("wq", "wk", "wv"):
        for part in ("h", "l"):
            w_ext[wname + "8" + part] = nc.declare_dram_parameter(
                wname + "8" + part, [HL, 128, EC, E], FP8, isOutput=False)
    out_ext = nc.declare_dram_parameter("out", [S, E], F32, isOutput=True)

    with tile.TileContext(nc) as tc:
        with tc.tile_pool(name="sb", bufs=1) as sb, \
             tc.tile_pool(name="sbt", bufs=1) as sbt, \
             tc.tile_pool(name="ps", bufs=1, space="PSUM") as ps, \
             tc.tile_pool(name="dram", bufs=1, space="DRAM") as dram:

            ones32 = sb.tile([128, 1], F32, name="ones32")
            nc.any.memset(ones32[:], 1.0)
            ident128 = sb.tile([128, 128], F32, name="ident128")
            masks.make_identity(nc, ident128[:])

            xu_tiles = []
            for sbk in range(SB):
                xu = sbt.tile([128, 4 * E], F16, name="xu", tag="xu", bufs=4)
                for hv in range(2):
                    nc.sync.dma_start(
                        xu[:, 2 * hv * E:2 * (hv + 1) * E]
                        .rearrange("p (t e) -> p t e", t=2),
                        x_ext[sbk * 512 + hv * 256:sbk * 512 + (hv + 1) * 256, :]
                        .rearrange("(t p) e -> p t e", p=128))
                xu_tiles.append(xu)

            pools = dict(sb=sb, sbt=sbt, ps=ps, dram=dram)
            _build_body(nc, tc, pools, xu_tiles, w_ext, ones32, ident128,
                        out_ext)

    nc.compile()
    return nc


def _build_body(nc, tc, pools, xu_tiles, w_ext, ones32, ident128, out_ext):
    sb, sbt, ps, dram = pools['sb'], pools['sbt'], pools['ps'], pools['dram']

    def big_ps(tag="pp", bufs=2):
        return ps.tile([128, 512], F32, name=tag, tag=tag, bufs=bufs)

    def sm_ps():
        # shared small-psum ring: pstk/pstq/pv/wops all [128, 256]
        return ps.tile([128, 256], F32, name="sm", tag="sm", bufs=2)

    # ============ LayerNorm pieces (called per s-block) ============
    xn = {}
    for ec in range(EC):
        for sbk in range(SB):
            xn[ec, sbk] = sb.tile([128, 512], F32R, name=f"xn_{ec}_{sbk}")
    xn8 = {}
    for part in ("h", "l"):
        for sbk in range(SB):
            xn8[part, sbk] = sb.tile([128, 2, 512], FP8,
                                     name=f"xn8{part}_{sbk}")

    def emit_xn8(sbk):
        """fp8 hi/lo of 8*xnT for one s-block (feeds head>=2 projections);
        the x8 keeps the lo residuals out of fp8e4's subnormal range.
        Emitted during head 0's prefetch slots where ACT/DVE have slack."""
        for ec in range(EC):
            nc.scalar.activation(xn8["h", sbk][:, ec, :], xn[ec, sbk][:],
                                 AF.Identity, scale=8.0)
            nc.vector.scalar_tensor_tensor(xn8["l", sbk][:, ec, :],
                                           xn[ec, sbk][:], 8.0,
                                           xn8["h", sbk][:, ec, :],
                                           OP.mult, OP.subtract)

    def emit_ln(sbk):
        xu = xu_tiles[sbk]
        st6 = sbt.tile([128, 4, 6], F32, name="st6", tag="st6", bufs=2)
        mv = sbt.tile([128, 4, 2], F32, name="mv", tag="mv", bufs=2)
        vb = sbt.tile([128, 4], F32, name="vb", tag="vb", bufs=2)
        inv4 = sbt.tile([128, 4], F32, name="inv4", tag="inv4", bufs=2)
        for j in range(4):
            nc.vector.bn_stats(st6[:, j], xu[:, j * E:(j + 1) * E])
            nc.vector.bn_aggr(mv[:, j], st6[:, j])
        rcp = sbt.tile([128, 4], F32, name="rcp", tag="rcp", bufs=2)
        nc.vector.tensor_scalar_add(vb[:], mv[:, :, 1], EPS)
        with nc.allow_low_precision("~18-bit reciprocal + table sqrt is"
                                    " plenty for a LN scale"):
            nc.vector.reciprocal_approx_fast(rcp[:], vb[:])
        nc.scalar.activation(inv4[:], rcp[:], AF.Sqrt, scale=1.0)
        xnus = []
        for j in range(4):
            xnu = sbt.tile([128, E], F32, name="xnu", tag="xnu", bufs=6)
            nc.gpsimd.tensor_scalar(xnu[:], xu[:, j * E:(j + 1) * E],
                                    mv[:, j, 0:1], inv4[:, j:j + 1],
                                    OP.subtract, OP.mult)
            xnus.append(xnu)
        for ec in range(EC):
            pt2 = big_ps()
            for j in range(4):
                nc.tensor.transpose(pt2[:, j * 128:(j + 1) * 128],
                                    xnus[j][:, ec * 128:(ec + 1) * 128],
                                    ident128[:])
            if ec == 0:
                nc.scalar.copy(xn[ec, sbk][:], pt2[:])
            else:
                nc.vector.tensor_copy(xn[ec, sbk][:], pt2[:])

    def xn_col(ec, st):
        sbk, j = divmod(st, 4)
        return xn[ec, sbk][:, j * 128:(j + 1) * 128]

    SL = [slice(i * 512, (i + 1) * 512) for i in range(SB)]

    # ============ per-head state ============
    acc = sb.tile([128, ST * E], F32, name="acc")

    bounce_in = [dram.tile([S // 2, E], F32, name=f"bounce_in{i}",
                           tag=f"bin{i}", bufs=1) for i in range(2)]
    bounce_view = [b.rearrange("(t p) e -> p t e", p=128) for b in bounce_in]

    st_h = {}

    def new_head_state(h):
        w = {}
        names = ["wo"]
        if h < 2 or not FP8_PROJ:
            names += ["wk", "wq"]
            if h == 0 or not FP8_V:
                names.append("wv")
        for wname in names:
            wr = sbt.tile([128, EC * E], F32R, name=f"w_{wname}",
                          tag=f"w_{wname}", bufs=2)
            nc.sync.dma_start(wr[:], w_ext[wname][h])
            w[wname] = wr
        w8names = []
        if FP8_PROJ and h >= 2:
            w8names += ["wk", "wq", "wv"]
        elif FP8_V and h >= 1:
            w8names.append("wv")
        for wname in w8names:
            for part in ("h", "l"):
                w8 = sbt.tile([128, EC, E], FP8, name=f"w8_{wname}{part}",
                              tag=f"w8_{wname}{part}", bufs=2)
                nc.sync.dma_start(w8[:], w_ext[wname + "8" + part][h])
                w[wname + "8" + part] = w8
        st_h[h] = dict(w=w, khi={}, klo={}, qhi={}, qlo={}, vt={}, outT={},
                       biasq={}, eq2q={}, sqk={}, sqq={})

    def emit_projA(h, sbk, which):
        """f32r projection of K^T or Q^T for one s-block + fp8 hi/lo
        quantization + per-feature squares (head-0 K on ACT for the warmup
        critical path; otherwise reconstructed from hi+lo on the idle Pool
        engine, which cannot read PSUM)."""
        s = st_h[h]
        wr = s['w'].get('wk' if which == 'k' else 'wq')
        hi = sbt.tile([128, 2, 512], FP8, name=which + "hi", tag=which + "hi",
                      bufs=8)
        lo = sbt.tile([128, 2, 512], FP8, name=which + "lo", tag=which + "lo",
                      bufs=8)
        sqs = []
        for ft in range(EC):
            pp = big_ps()
            if h < 2 or not FP8_PROJ:
                for ec in range(EC):
                    o = ec * E + ft * 128
                    nc.tensor.matmul(pp[:], wr[:, o:o + 128],
                                     xn[ec, sbk][:],
                                     start=(ec == 0), stop=(ec == EC - 1))
            else:
                wn = 'wk' if which == 'k' else 'wq'
                w8h = s['w'][wn + '8h']
                w8l = s['w'][wn + '8l']
                o = ft * 128
                xh, xl = xn8["h", sbk][:], xn8["l", sbk][:]
                nc.tensor.matmul(pp[:], w8h[:, :, o:o + 128], xh,
                                 start=True, stop=False, perf_mode=DR)
                nc.tensor.matmul(pp[:], w8h[:, :, o:o + 128], xl,
                                 start=False, stop=False, perf_mode=DR)
                nc.tensor.matmul(pp[:], w8l[:, :, o:o + 128], xh,
                                 start=False, stop=True, perf_mode=DR)
            if FP8_PROJ and h >= 2:
                # fp8-weight projection left pp scaled by 512
                nc.vector.tensor_scalar_mul(hi[:, ft, :], pp[:], 1.0 / 512.0)
                nc.vector.scalar_tensor_tensor(lo[:, ft, :], pp[:],
                                               1.0 / 512.0, hi[:, ft, :],
                                               OP.mult, OP.subtract)
            else:
                if h == 0 and (which == 'k' or sbk <= 1):
                    nc.scalar.copy(hi[:, ft, :], pp[:])
                else:
                    nc.vector.tensor_copy(hi[:, ft, :], pp[:])
                nc.vector.tensor_tensor(lo[:, ft, :], pp[:], hi[:, ft, :],
                                        OP.subtract)
            sq = sbt.tile([128, 512], F32, name="sq", tag="sq", bufs=10)
            if h == 0 and which == 'k':
                nc.scalar.activation(sq[:], pp[:], AF.Square, scale=1.0)
            else:
                tsum = sbt.tile([128, 512], F32, name="tsum", tag="tsum",
                                bufs=2)
                nc.gpsimd.tensor_tensor(tsum[:], hi[:, ft, :], lo[:, ft, :],
                                        OP.add)
                nc.gpsimd.tensor_tensor(sq[:], tsum[:], tsum[:], OP.mult)
            sqs.append(sq)
        if which == 'k':
            s['khi'][sbk], s['klo'][sbk] = hi, lo
            s['sqk'][sbk] = sqs
        else:
            s['qhi'][sbk], s['qlo'][sbk] = hi, lo
            s['sqq'][sbk] = sqs

    def emit_bias(h, sbk, which):
        """k2/q2 per-partition columns via [128,1] fp32 ones-matmuls.
        Emitted a prefetch slot after emit_projA so the PE stream never
        waits on the Pool square chain."""
        s = st_h[h]
        sqs = (s['sqk'] if which == 'k' else s['sqq']).pop(sbk)
        pst = sm_ps()
        for j in range(4):
            for ft in range(EC):
                nc.tensor.matmul(pst[:, j:j + 1],
                                 sqs[ft][:, j * 128:(j + 1) * 128],
                                 ones32[:], start=(ft == 0),
                                 stop=(ft == EC - 1))
        if which == 'k':
            bq = sbt.tile([128, 4], F32, name="biasq", tag="biasq", bufs=8)
            nc.vector.tensor_scalar_mul(bq[:], pst[:, 0:4], -0.5)
            s['biasq'][sbk] = bq
        else:
            eq = sbt.tile([128, 4], F32, name="eq2q", tag="eq2q", bufs=8)
            nc.scalar.activation(eq[:], pst[:, 0:4], AF.Exp, scale=-0.5)
            s['eq2q'][sbk] = eq

    def emit_proj(h, sbk, which):
        emit_projA(h, sbk, which)
        emit_bias(h, sbk, which)

    def emit_v(h, sbk):
        s = st_h[h]
        wv = s['w'].get('wv')
        for st in range(sbk * 4, sbk * 4 + 4):
            xsb, xj = divmod(st, 4)
            csl = slice(xj * 128, (xj + 1) * 128)
            pv = sm_ps()
            if not ((FP8_V and h >= 1) or (FP8_PROJ and h >= 2)):
                for ec in range(EC):
                    nc.tensor.matmul(pv[:], xn_col(ec, st),
                                     wv[:, ec * E:(ec + 1) * E],
                                     start=(ec == 0), stop=(ec == EC - 1))
            else:
                wvh, wvl = s['w']['wv8h'], s['w']['wv8l']
                nc.tensor.matmul(pv[:], xn8["h", xsb][:, :, csl], wvh[:],
                                 start=True, stop=False, perf_mode=DR)
                nc.tensor.matmul(pv[:], xn8["h", xsb][:, :, csl], wvl[:],
                                 start=False, stop=False, perf_mode=DR)
                nc.tensor.matmul(pv[:], xn8["l", xsb][:, :, csl], wvh[:],
                                 start=False, stop=True, perf_mode=DR)
            v = sbt.tile([128, E], F32R, name="vt", tag="vt", bufs=24)
            if (FP8_V and h >= 1) or (FP8_PROJ and h >= 2):
                nc.vector.tensor_scalar_mul(v[:], pv[:], 1.0 / 512.0)
            else:
                nc.vector.tensor_copy(v[:], pv[:])
            s['vt'][st] = v

    sc_pre = {}

    def emit_score_tile(h, sbk, tt):
        """Scores + exp for one [128t, 512s] tile (3 fp8 DoubleRow matmuls
        + one biased ACT exp)."""
        s = st_h[h]
        tb, tj = divmod(tt, 4)
        csl = slice(tj * 128, (tj + 1) * 128)
        kh = s['khi'][tb][:, :, csl]
        kl = s['klo'][tb][:, :, csl]
        qh, ql = s['qhi'][sbk][:], s['qlo'][sbk][:]
        stps = big_ps(tag="stps", bufs=2)
        nc.tensor.matmul(stps[:], kh, qh, start=True, stop=False,
                         perf_mode=DR)
        nc.tensor.matmul(stps[:], kh, ql, start=False, stop=False,
                         perf_mode=DR)
        nc.tensor.matmul(stps[:], kl, qh, start=False, stop=True,
                         perf_mode=DR)
        sc = sbt.tile([128, 512], F32R, name="sc", tag="sc", bufs=12)
        nc.scalar.activation(sc[:], stps[:], AF.Exp,
                             bias=s['biasq'][tb][:, tj:tj + 1], scale=1.0)
        return sc

    def emit_main(h, sbk, warm_next=None):
        """Main loop for one s-block.  warm_next=(h', sbk') pre-emits that
        block's first SKEW score tiles during this block's AV-only tail so
        the next main never waits on the exp chain."""
        s = st_h[h]
        vt = s['vt']
        ops = [big_ps(tag="ov", bufs=2) for _ in range(EC)]
        sc_q = {}
        # head 0 s-block 0 runs more scores ahead of the AVs so the PE
        # stream is not blocked by the warmup DVE/ACT quantization backlog.
        SKEW = 4 if (h == 0 and sbk == 0) else 2
        for tt in range(ST + SKEW):
            if tt < ST:
                if (h, sbk, tt) in sc_pre:
                    sc_q[tt] = sc_pre.pop((h, sbk, tt))
                else:
                    sc_q[tt] = emit_score_tile(h, sbk, tt)
            elif warm_next is not None:
                wh, wsbk = warm_next
                wt = tt - ST
                if wt < 2:
                    sc_pre[(wh, wsbk, wt)] = emit_score_tile(wh, wsbk, wt)
            if tt >= SKEW:
                pv_tt = tt - SKEW
                sc_prev = sc_q.pop(pv_tt)
                for ft in range(EC):
                    nc.tensor.matmul(ops[ft][:],
                                     vt[pv_tt][:, ft * 128:(ft + 1) * 128],
                                     sc_prev[:],
                                     start=(pv_tt == 0), stop=(pv_tt == ST - 1))
        for ft in range(EC):
            o = sbt.tile([128, 512], F32R, name="outT", tag="outT", bufs=8)
            nc.vector.tensor_copy(o[:, 0:256], ops[ft][:, 0:256])
            nc.scalar.copy(o[:, 256:512], ops[ft][:, 256:512])
            s['outT'][ft, sbk] = o

    def emit_wo(h, sbk):
        s = st_h[h]
        wo = s['w']['wo']
        for st in range(sbk * 4, sbk * 4 + 4):
            j = st % 4
            wops = sm_ps()
            for ft in range(EC):
                nc.tensor.matmul(wops[:],
                                 s['outT'][ft, sbk][:, j * 128:(j + 1) * 128],
                                 wo[:, ft * E:(ft + 1) * E],
                                 start=(ft == 0), stop=(ft == EC - 1))
            asl = acc[:, st * E:(st + 1) * E]
            qb, qj = divmod(st, 4)
            eqcol = s['eq2q'][qb][:, qj:qj + 1]
            if h == 0:
                nc.vector.tensor_scalar(asl, wops[:], eqcol, None, OP.mult)
            else:
                nc.vector.scalar_tensor_tensor(asl, wops[:], eqcol,
                                               asl, OP.mult, OP.add)
        if h == N_HEADS_BUILD - 1:
            half, sth = divmod(sbk * 4, 8)
            tgt = bounce_view[half][:, sth:sth + 4, :]
            if sbk < SB - 1:
                nc.sync.dma_start(
                    tgt,
                    acc[:, sbk * 4 * E:(sbk + 1) * 4 * E]
                    .rearrange("p (t e) -> p t e", e=E))
            else:
                nc.sync.dma_start(
                    tgt[:, 0:2, :],
                    acc[:, sbk * 4 * E:(sbk * 4 + 2) * E]
                    .rearrange("p (t e) -> p t e", e=E))
                for ei in range(2, 4):
                    st0 = sbk * 4 + ei
                    nc.sync.dma_start(
                        tgt[:, ei:ei + 1, :],
                        acc[:, st0 * E:(st0 + 1) * E]
                        .rearrange("p (t e) -> p t e", e=E))

    # ============ emission schedule ============
    if N_HEADS_BUILD == 0:
        nc.any.memset(acc[:], 0.0)
    else:
        # LN interleaved with head-0 K projections: main(0,0) needs K/k2 of
        # all four s-blocks, so those quantization chains are the warmup
        # critical path (K squares on ACT there, bias inline).
        for sbk in range(SB):
            emit_ln(sbk)
            if sbk == 0:
                new_head_state(0)
            emit_proj(0, sbk, 'k')
        emit_projA(0, 0, 'q')
        emit_projA(0, 1, 'q')
        for sbk in range(SB):
            emit_v(0, sbk)
        emit_bias(0, 0, 'q')
        emit_bias(0, 1, 'q')

    for h in range(N_HEADS_BUILD):
        nxt = h + 1
        if nxt < N_HEADS_BUILD:
            new_head_state(nxt)
        for sbk in range(SB):
            if sbk < SB - 1:
                wn = (h, sbk + 1)
            elif nxt < N_HEADS_BUILD:
                wn = (nxt, 0)
            else:
                wn = None
            emit_main(h, sbk, warm_next=wn)
            if h == 0:
                # finish head 0's own pieces
                if sbk == 0:
                    emit_projA(0, 2, 'q')
                    emit_projA(0, 3, 'q')
                elif sbk == 1:
                    emit_bias(0, 2, 'q')
                elif sbk == 2:
                    emit_bias(0, 3, 'q')
            if (FP8_V or FP8_PROJ) and h == 0:
                emit_xn8(sbk)
            if nxt < N_HEADS_BUILD:
                if sbk == 0:
                    if h > 0:
                        emit_bias(h, 2, 'q')
                    emit_projA(nxt, 0, 'k')
                    emit_projA(nxt, 1, 'k')
                elif sbk == 1:
                    if h > 0:
                        emit_bias(h, 3, 'q')
                    emit_projA(nxt, 2, 'k')
                    emit_projA(nxt, 3, 'k')
                    emit_bias(nxt, 0, 'k')
                    emit_bias(nxt, 1, 'k')
                elif sbk == 2:
                    emit_bias(nxt, 2, 'k')
                    emit_bias(nxt, 3, 'k')
                    emit_projA(nxt, 0, 'q')
                    emit_projA(nxt, 1, 'q')
                    emit_v(nxt, 0)
                    emit_v(nxt, 1)
                else:
                    emit_bias(nxt, 0, 'q')
                    emit_bias(nxt, 1, 'q')
                    emit_projA(nxt, 2, 'q')
                    emit_projA(nxt, 3, 'q')
                    emit_v(nxt, 2)
                    emit_v(nxt, 3)
            else:
                if sbk == 0:
                    emit_bias(h, 2, 'q')
                elif sbk == 1:
                    emit_bias(h, 3, 'q')
            emit_wo(h, sbk)
        if h > 0:
            st_h.pop(h - 1, None)

    if N_HEADS_BUILD == 0:
        for half in range(2):
            nc.sync.dma_start(
                bounce_view[half][:, :, :],
                acc[:, half * 8 * E:(half + 1) * 8 * E]
                .rearrange("p (t e) -> p t e", e=E))

    # ==== AllReduce over batch pair + store (quarters; last one split) ====
    bos = [dram.tile([S // 2, E], F32, name=f"bounce_out{i}",
                     tag=f"bout{i}", bufs=1) for i in range(2)]
    chunks = [(0, 0, 512), (0, 512, 512), (1, 0, 512), (1, 512, 256),
              (1, 768, 128), (1, 896, 128)]
    for half, r0, rn in chunks:
        rsl = slice(r0, r0 + rn)
        o0 = half * (S // 2) + r0
        osl = out_ext[o0:o0 + rn, :]
        if NO_COLL:
            nc.sync.dma_start(osl, bounce_in[half][rsl, :])
        else:
            nc.gpsimd.collective_compute(
                "AllReduce", OP.add,
                replica_groups=[[0, 1], [2, 3], [4, 5], [6, 7]],
                ins=[bounce_in[half][rsl, :].opt()],
                outs=[bos[half][rsl, :].opt()],
            )
            nc.sync.dma_start(osl, bos[half][rsl, :])


# ================= host side =================

def prep_inputs(x, ln_scale, W_q, W_k, W_v, W_o, gamma):
    """Build per-core input maps."""
    x = np.asarray(x, np.float32)
    ln_scale = np.asarray(ln_scale, np.float32)
    W_q = np.asarray(W_q, np.float32)
    W_k = np.asarray(W_k, np.float32)
    W_v = np.asarray(W_v, np.float32)
    W_o = np.asarray(W_o, np.float32)
    gamma = np.asarray(gamma, np.float32).reshape(H)

    in_maps = []
    for c in range(N_CORES):
        b = c // 2
        h0 = HL * (c % 2)
        hs = list(range(h0, h0 + HL))
        g = gamma[hs]
        s2g = np.sqrt(2.0 * g).astype(np.float32)
        wq = (W_q[hs] * ln_scale[None, :, None] * s2g[:, None, None])
        wk = (W_k[hs] * ln_scale[None, :, None] * s2g[:, None, None])
        wv = (W_v[hs] * ln_scale[None, :, None])
        def _lay(w):   # [HL, E_in(=EC*128), E] -> [HL, 128, EC*E]
            return np.ascontiguousarray(
                w.reshape(HL, EC, 128, E).transpose(0, 2, 1, 3).reshape(HL, 128, EC * E))
        import ml_dtypes
        FP8NP = ml_dtypes.float8_e4m3

        def _split8(w):   # [HL, 128, EC*E] -> fp8 hi/lo [HL, 128, EC, E]
            # x64 lifts the ~0.03-rms folded weights out of fp8e4's
            # subnormal floor; the PSUM drains descale by 1/64.
            w4 = w.reshape(HL, 128, EC, E) * np.float32(64.0)
            hi = w4.astype(FP8NP)
            lo = (w4 - hi.astype(np.float32)).astype(FP8NP)
            return np.ascontiguousarray(hi), np.ascontiguousarray(lo)

        wq = _lay(wq)
        wk = _lay(wk)
        wv = _lay(wv)
        wo = _lay(np.stack([W_o[:, 256 * h:256 * (h + 1)].T.copy() for h in hs]))
        wq8h, wq8l = _split8(wq)
        wk8h, wk8l = _split8(wk)
        wv8h, wv8l = _split8(wv)
        in_maps.append({
            "x": np.ascontiguousarray(x[b]).astype(np.float16),
            "wq": np.ascontiguousarray(wq),
            "wk": np.ascontiguousarray(wk),
            "wv": np.ascontiguousarray(wv),
            "wo": np.ascontiguousarray(wo),
            "wq8h": wq8h, "wq8l": wq8l,
            "wk8h": wk8h, "wk8l": wk8l,
            "wv8h": wv8h, "wv8l": wv8l,
        })
    return in_maps


def assemble_output(results):
    out = np.empty((B, S, E), np.float32)
    for b in range(B):
        out[b] = results[2 * b]["out"]
    return out


_NC_CACHE = {}


def _get_nc():
    if 'nc' not in _NC_CACHE:
        _NC_CACHE['nc'] = build_kernel(R=1, debug=False)
    return _NC_CACHE['nc']


def kernel(x, e=None, p=None, ln_scale=None, W_q=None, W_k=None, W_v=None,
           W_o=None, gamma=None, **_unused):
    """Full-input entry point. e and p are unused by the reference network
    (use_ppe=False config); they are accepted and ignored."""
    in_maps = prep_inputs(x, ln_scale, W_q, W_k, W_v, W_o, gamma)
    nc = _get_nc()
    res = run_bass_kernel_spmd(nc, in_maps, core_ids=list(range(N_CORES)))
    return assemble_output(res.results)


# revision 44
# speedup vs baseline: 1.0013x; 1.0005x over previous
"""RBF-kernel attention (nn_Attention_76081050682051) on 8 TRN2 NeuronCores.

Self-contained Bass/Tile kernel. `kernel(**inputs)` takes the FULL unsharded
inputs of reference.setup_inputs() and returns the FULL [4, 2048, 256] f32
output.

Sharding (B x tensor-parallel heads): core c -> batch b = c//2, heads
[4*(c%2), 4*(c%2)+4); pairwise AllReduce ([0,1],[2,3],[4,5],[6,7]) combines
the two half-head partial outputs of each batch after the W_o projection.

Device math (all numerics device-verified; ~5.1e-3 rel err vs 2e-2 gate):
  x ships as fp16 (N(0,1) data; 10-bit mantissa costs ~0.1% in scores).
  LayerNorm per-partition stats via bn_stats/bn_aggr; rsqrt via DVE
  reciprocal_approx_fast + one ACT Sqrt; normalize on GPSIMD; xnT built by
  PE transposes batched 4-per-psum-tile.  Weights are DMA'd straight into
  f32r tiles (host arrays are plain f32; walrus accepts DMA producers).
  Per head: K'T/Q'T = (folded W).T @ xnT in f32r, then quantized to fp8e4
  hi+lo pairs.  The QK^T scores matmul runs as 3 fp8 DoubleRow matmuls per
  [128,512] tile (hi*hi + hi*lo + lo*hi, contraction 256 per instr at 0.5
  cyc/row) - 1.33x the f32r rate at ~0.4% error.  V projections for heads
  1-3 are also 3-term fp8 DoubleRow (xn8 = fp8(8*xn) hi/lo converted during
  head 0's window, W8 = fp8(64*W) from the host, 1/512 descale in the vt
  drain); safe because V has no exp downstream (see FP8_V note).
  k2/q2 row sums: per-feature squares (ACT Square on the warmup path, Pool
  hi+lo reconstruction steady-state, bias matmuls deferred one prefetch
  slot) then [128,1]-output plain-fp32 ones-matmuls put k2/q2 directly on
  partitions - no DRAM roundtrip.
  scoresT[t,s] = exp(qk'[t,s] - k2'[t]/2) via one ACT exp per [128,512]
  tile (per-partition bias); exp(-q2'[s]/2) is applied after W_o as a
  per-partition scale.  outT = V.T @ scoresT accumulates over t in f32r
  (fp8 needs >=8 mantissa bits here and outT inherits the unnormalized
  scores' e^+-17 per-column range - both over budget).  W_o runs on outT
  column slices; partial outputs AllReduce within each batch pair at
  quarter granularity (last quarter split further to shrink the tail).
  Emission is software-pipelined across heads: LayerNorm interleaves with
  head-0 K projections, each s-block main pre-emits the next block's first
  two score tiles during its AV-only tail (the exp chain never gates the
  in-order PE stream at boundaries), and projection/V prefetch for head
  h+1 rides inside head h's mains.
"""
import sys
sys.path.insert(0, '/opt/trn_rl_repo')
import numpy as np
from concourse import bass, bacc, tile, mybir, masks
from concourse.bass_utils import run_bass_kernel_spmd

F32 = mybir.dt.float32
F32R = mybir.dt.float32r
F16 = mybir.dt.float16
FP8 = mybir.dt.float8e4
AF = mybir.ActivationFunctionType
OP = mybir.AluOpType
DR = mybir.MatmulPerfMode.DoubleRow

B, S, E, H = 4, 2048, 256, 8
HL = 4          # heads per core
EC = 2          # e chunks of 128
SB = 4          # s blocks of 512
ST = 16         # s/t tiles of 128
N_CORES = 8
EPS = 1e-5

NO_COLL = False
N_HEADS_BUILD = HL
# fp8 DoubleRow projections for heads 2-3: ~3.7us faster but adds
# ~7.6e-3 relative error.  The ~1.2e-3 per-element error of the 3-term
# hi/lo product (dropped lo*lo + both lo-requantizations) amplifies by
# sqrt(E)=16 through the qk contraction into ~0.5% score error per
# affected head, plus correlated k2/q2 bias errors -- fundamental to the
# split, and a 4th term would erase the PE savings.  Disabled to keep a
# 4x margin to the 2e-2 correctness gate.
FP8_PROJ = False
# fp8 DoubleRow V projections for heads 1-3 ARE safe: V has no exp
# downstream, so its ~1.2e-3 per-element hi/lo error reaches the output
# at ~1.2e-3 (no sqrt(E) amplification), and the vt drain stays one DVE
# op (descale instead of copy).  Head 0's V runs before the xn8
# conversion window, so it stays f32r.
FP8_V = True


def build_kernel(R=1, debug=False):
    nc = bacc.Bacc("TRN2", target_bir_lowering=False, debug=False,
                   num_devices=N_CORES)

    x_ext = nc.declare_dram_parameter("x", [S, E], F16, isOutput=False)
    w_ext = {}
    for wname in ("wq", "wk", "wv", "wo"):
        w_ext[wname] = nc.declare_dram_parameter(wname, [HL, 128, EC * E],
                                                 F32R, isOutput=False)
    for wname in ("wq", "wk", "wv"):
        for part in ("h", "l"):
            w_ext[wname + "8" + part] = nc.declare_dram_parameter(
                wname + "8" + part, [HL, 128, EC, E], FP8, isOutput=False)
    out_ext = nc.declare_dram_parameter("out", [S, E], F32, isOutput=True)

    with tile.TileContext(nc) as tc:
        with tc.tile_pool(name="sb", bufs=1) as sb, \
             tc.tile_pool(name="sbt", bufs=1) as sbt, \
             tc.tile_pool(name="ps", bufs=1, space="PSUM") as ps, \
             tc.tile_pool(name="dram", bufs=1, space="DRAM") as dram:

            ones32 = sb.tile([128, 1], F32, name="ones32")
            nc.any.memset(ones32[:], 1.0)
            ident128 = sb.tile([128, 128], F32, name="ident128")
            masks.make_identity(nc, ident128[:])

            xu_tiles = []
            for sbk in range(SB):
                xu = sbt.tile([128, 4 * E], F16, name="xu", tag="xu", bufs=4)
                for hv in range(2):
                    nc.sync.dma_start(
                        xu[:, 2 * hv * E:2 * (hv + 1) * E]
                        .rearrange("p (t e) -> p t e", t=2),
                        x_ext[sbk * 512 + hv * 256:sbk * 512 + (hv + 1) * 256, :]
                        .rearrange("(t p) e -> p t e", p=128))
                xu_tiles.append(xu)

            pools = dict(sb=sb, sbt=sbt, ps=ps, dram=dram)
            _build_body(nc, tc, pools, xu_tiles, w_ext, ones32, ident128,
                        out_ext)

    nc.compile()
    return nc


def _build_body(nc, tc, pools, xu_tiles, w_ext, ones32, ident128, out_ext):
    sb, sbt, ps, dram = pools['sb'], pools['sbt'], pools['ps'], pools['dram']

    def big_ps(tag="pp", bufs=2):
        return ps.tile([128, 512], F32, name=tag, tag=tag, bufs=bufs)

    def sm_ps():
        # shared small-psum ring: pstk/pstq/pv/wops all [128, 256]
        return ps.tile([128, 256], F32, name="sm", tag="sm", bufs=2)

    # ============ LayerNorm pieces (called per s-block) ============
    xn = {}
    for ec in range(EC):
        for sbk in range(SB):
            xn[ec, sbk] = sb.tile([128, 512], F32R, name=f"xn_{ec}_{sbk}")
    xn8 = {}
    for part in ("h", "l"):
        for sbk in range(SB):
            xn8[part, sbk] = sb.tile([128, 2, 512], FP8,
                                     name=f"xn8{part}_{sbk}")

    def emit_xn8(sbk):
        """fp8 hi/lo of 8*xnT for one s-block (feeds head>=2 projections);
        the x8 keeps the lo residuals out of fp8e4's subnormal range.
        Emitted during head 0's prefetch slots where ACT/DVE have slack."""
        for ec in range(EC):
            nc.scalar.activation(xn8["h", sbk][:, ec, :], xn[ec, sbk][:],
                                 AF.Identity, scale=8.0)
            nc.vector.scalar_tensor_tensor(xn8["l", sbk][:, ec, :],
                                           xn[ec, sbk][:], 8.0,
                                           xn8["h", sbk][:, ec, :],
                                           OP.mult, OP.subtract)

    def emit_ln(sbk):
        xu = xu_tiles[sbk]
        st6 = sbt.tile([128, 4, 6], F32, name="st6", tag="st6", bufs=2)
        mv = sbt.tile([128, 4, 2], F32, name="mv", tag="mv", bufs=2)
        vb = sbt.tile([128, 4], F32, name="vb", tag="vb", bufs=2)
        inv4 = sbt.tile([128, 4], F32, name="inv4", tag="inv4", bufs=2)
        for j in range(4):
            nc.vector.bn_stats(st6[:, j], xu[:, j * E:(j + 1) * E])
            nc.vector.bn_aggr(mv[:, j], st6[:, j])
        rcp = sbt.tile([128, 4], F32, name="rcp", tag="rcp", bufs=2)
        nc.vector.tensor_scalar_add(vb[:], mv[:, :, 1], EPS)
        with nc.allow_low_precision("~18-bit reciprocal + table sqrt is"
                                    " plenty for a LN scale"):
            nc.vector.reciprocal_approx_fast(rcp[:], vb[:])
        nc.scalar.activation(inv4[:], rcp[:], AF.Sqrt, scale=1.0)
        xnus = []
        for j in range(4):
            xnu = sbt.tile([128, E], F32, name="xnu", tag="xnu", bufs=6)
            nc.gpsimd.tensor_scalar(xnu[:], xu[:, j * E:(j + 1) * E],
                                    mv[:, j, 0:1], inv4[:, j:j + 1],
                                    OP.subtract, OP.mult)
            xnus.append(xnu)
        for ec in range(EC):
            pt2 = big_ps()
            for j in range(4):
                nc.tensor.transpose(pt2[:, j * 128:(j + 1) * 128],
                                    xnus[j][:, ec * 128:(ec + 1) * 128],
                                    ident128[:])
            if ec == 0:
                nc.scalar.copy(xn[ec, sbk][:], pt2[:])
            else:
                nc.vector.tensor_copy(xn[ec, sbk][:], pt2[:])

    def xn_col(ec, st):
        sbk, j = divmod(st, 4)
        return xn[ec, sbk][:, j * 128:(j + 1) * 128]

    SL = [slice(i * 512, (i + 1) * 512) for i in range(SB)]

    # ============ per-head state ============
    acc = sb.tile([128, ST * E], F32, name="acc")

    bounce_in = [dram.tile([S // 2, E], F32, name=f"bounce_in{i}",
                           tag=f"bin{i}", bufs=1) for i in range(2)]
    bounce_view = [b.rearrange("(t p) e -> p t e", p=128) for b in bounce_in]

    st_h = {}

    def new_head_state(h):
        w = {}
        names = ["wo"]
        if h < 2 or not FP8_PROJ:
            names += ["wk", "wq"]
            if h == 0 or not FP8_V:
                names.append("wv")
        for wname in names:
            wr = sbt.tile([128, EC * E], F32R, name=f"w_{wname}",
                          tag=f"w_{wname}", bufs=2)
            nc.sync.dma_start(wr[:], w_ext[wname][h])
            w[wname] = wr
        w8names = []
        if FP8_PROJ and h >= 2:
            w8names += ["wk", "wq", "wv"]
        elif FP8_V and h >= 1:
            w8names.append("wv")
        for wname in w8names:
            for part in ("h", "l"):
                w8 = sbt.tile([128, EC, E], FP8, name=f"w8_{wname}{part}",
                              tag=f"w8_{wname}{part}", bufs=2)
                nc.sync.dma_start(w8[:], w_ext[wname + "8" + part][h])
                w[wname + "8" + part] = w8
        st_h[h] = dict(w=w, khi={}, klo={}, qhi={}, qlo={}, vt={}, outT={},
                       biasq={}, eq2q={}, sqk={}, sqq={})

    def emit_projA(h, sbk, which):
        """f32r projection of K^T or Q^T for one s-block + fp8 hi/lo
        quantization + per-feature squares (head-0 K on ACT for the warmup
        critical path; otherwise reconstructed from hi+lo on the idle Pool
        engine, which cannot read PSUM)."""
        s = st_h[h]
        wr = s['w'].get('wk' if which == 'k' else 'wq')
        hi = sbt.tile([128, 2, 512], FP8, name=which + "hi", tag=which + "hi",
                      bufs=8)
        lo = sbt.tile([128, 2, 512], FP8, name=which + "lo", tag=which + "lo",
                      bufs=8)
        sqs = []
        for ft in range(EC):
            pp = big_ps()
            if h < 2 or not FP8_PROJ:
                for ec in range(EC):
                    o = ec * E + ft * 128
                    nc.tensor.matmul(pp[:], wr[:, o:o + 128],
                                     xn[ec, sbk][:],
                                     start=(ec == 0), stop=(ec == EC - 1))
            else:
                wn = 'wk' if which == 'k' else 'wq'
                w8h = s['w'][wn + '8h']
                w8l = s['w'][wn + '8l']
                o = ft * 128
                xh, xl = xn8["h", sbk][:], xn8["l", sbk][:]
                nc.tensor.matmul(pp[:], w8h[:, :, o:o + 128], xh,
                                 start=True, stop=False, perf_mode=DR)
                nc.tensor.matmul(pp[:], w8h[:, :, o:o + 128], xl,
                                 start=False, stop=False, perf_mode=DR)
                nc.tensor.matmul(pp[:], w8l[:, :, o:o + 128], xh,
                                 start=False, stop=True, perf_mode=DR)
            if FP8_PROJ and h >= 2:
                # fp8-weight projection left pp scaled by 512
                nc.vector.tensor_scalar_mul(hi[:, ft, :], pp[:], 1.0 / 512.0)
                nc.vector.scalar_tensor_tensor(lo[:, ft, :], pp[:],
                                               1.0 / 512.0, hi[:, ft, :],
                                               OP.mult, OP.subtract)
            else:
                if h == 0 and (which == 'k' or sbk <= 1):
                    nc.scalar.copy(hi[:, ft, :], pp[:])
                else:
                    nc.vector.tensor_copy(hi[:, ft, :], pp[:])
                nc.vector.tensor_tensor(lo[:, ft, :], pp[:], hi[:, ft, :],
                                        OP.subtract)
            sq = sbt.tile([128, 512], F32, name="sq", tag="sq", bufs=10)
            if h == 0 and which == 'k':
                nc.scalar.activation(sq[:], pp[:], AF.Square, scale=1.0)
            else:
                tsum = sbt.tile([128, 512], F32, name="tsum", tag="tsum",
                                bufs=2)
                nc.gpsimd.tensor_tensor(tsum[:], hi[:, ft, :], lo[:, ft, :],
                                        OP.add)
                nc.gpsimd.tensor_tensor(sq[:], tsum[:], tsum[:], OP.mult)
            sqs.append(sq)
        if which == 'k':
            s['khi'][sbk], s['klo'][sbk] = hi, lo
            s['sqk'][sbk] = sqs
        else:
            s['qhi'][sbk], s['qlo'][sbk] = hi, lo
            s['sqq'][sbk] = sqs

    def emit_bias(h, sbk, which):
        """k2/q2 per-partition columns via [128,1] fp32 ones-matmuls.
        Emitted a prefetch slot after emit_projA so the PE stream never
        waits on the Pool square chain."""
        s = st_h[h]
        sqs = (s['sqk'] if which == 'k' else s['sqq']).pop(sbk)
        pst = sm_ps()
        for j in range(4):
            for ft in range(EC):
                nc.tensor.matmul(pst[:, j:j + 1],
                                 sqs[ft][:, j * 128:(j + 1) * 128],
                                 ones32[:], start=(ft == 0),
                                 stop=(ft == EC - 1))
        if which == 'k':
            bq = sbt.tile([128, 4], F32, name="biasq", tag="biasq", bufs=8)
            nc.vector.tensor_scalar_mul(bq[:], pst[:, 0:4], -0.5)
            s['biasq'][sbk] = bq
        else:
            eq = sbt.tile([128, 4], F32, name="eq2q", tag="eq2q", bufs=8)
            nc.scalar.activation(eq[:], pst[:, 0:4], AF.Exp, scale=-0.5)
            s['eq2q'][sbk] = eq

    def emit_proj(h, sbk, which):
        emit_projA(h, sbk, which)
        emit_bias(h, sbk, which)

    def emit_v(h, sbk):
        s = st_h[h]
        wv = s['w'].get('wv')
        for st in range(sbk * 4, sbk * 4 + 4):
            xsb, xj = divmod(st, 4)
            csl = slice(xj * 128, (xj + 1) * 128)
            pv = sm_ps()
            if not ((FP8_V and h >= 1) or (FP8_PROJ and h >= 2)):
                for ec in range(EC):
                    nc.tensor.matmul(pv[:], xn_col(ec, st),
                                     wv[:, ec * E:(ec + 1) * E],
                                     start=(ec == 0), stop=(ec == EC - 1))
            else:
                wvh, wvl = s['w']['wv8h'], s['w']['wv8l']
                nc.tensor.matmul(pv[:], xn8["h", xsb][:, :, csl], wvh[:],
                                 start=True, stop=False, perf_mode=DR)
                nc.tensor.matmul(pv[:], xn8["h", xsb][:, :, csl], wvl[:],
                                 start=False, stop=False, perf_mode=DR)
                nc.tensor.matmul(pv[:], xn8["l", xsb][:, :, csl], wvh[:],
                                 start=False, stop=True, perf_mode=DR)
            v = sbt.tile([128, E], F32R, name="vt", tag="vt", bufs=24)
            if (FP8_V and h >= 1) or (FP8_PROJ and h >= 2):
                nc.vector.tensor_scalar_mul(v[:], pv[:], 1.0 / 512.0)
            else:
                nc.vector.tensor_copy(v[:], pv[:])
            s['vt'][st] = v

    sc_pre = {}

    def emit_score_tile(h, sbk, tt):
        """Scores + exp for one [128t, 512s] tile (3 fp8 DoubleRow matmuls
        + one biased ACT exp)."""
        s = st_h[h]
        tb, tj = divmod(tt, 4)
        csl = slice(tj * 128, (tj + 1) * 128)
        kh = s['khi'][tb][:, :, csl]
        kl = s['klo'][tb][:, :, csl]
        qh, ql = s['qhi'][sbk][:], s['qlo'][sbk][:]
        stps = big_ps(tag="stps", bufs=2)
        nc.tensor.matmul(stps[:], kh, qh, start=True, stop=False,
                         perf_mode=DR)
        nc.tensor.matmul(stps[:], kh, ql, start=False, stop=False,
                         perf_mode=DR)
        nc.tensor.matmul(stps[:], kl, qh, start=False, stop=True,
                         perf_mode=DR)
        sc = sbt.tile([128, 512], F32R, name="sc", tag="sc", bufs=12)
        nc.scalar.activation(sc[:], stps[:], AF.Exp,
                             bias=s['biasq'][tb][:, tj:tj + 1], scale=1.0)
        return sc

    def emit_main(h, sbk, warm_next=None):
        """Main loop for one s-block.  warm_next=(h', sbk') pre-emits that
        block's first SKEW score tiles during this block's AV-only tail so
        the next main never waits on the exp chain."""
        s = st_h[h]
        vt = s['vt']
        ops = [big_ps(tag="ov", bufs=2) for _ in range(EC)]
        sc_q = {}
        # head 0 s-block 0 runs more scores ahead of the AVs so the PE
        # stream is not blocked by the warmup DVE/ACT quantization backlog.
        SKEW = 2
        for tt in range(ST + SKEW):
            if tt < ST:
                if (h, sbk, tt) in sc_pre:
                    sc_q[tt] = sc_pre.pop((h, sbk, tt))
                else:
                    sc_q[tt] = emit_score_tile(h, sbk, tt)
            elif warm_next is not None:
                wh, wsbk = warm_next
                wt = tt - ST
                if wt < 2:
                    sc_pre[(wh, wsbk, wt)] = emit_score_tile(wh, wsbk, wt)
            if tt >= SKEW:
                pv_tt = tt - SKEW
                sc_prev = sc_q.pop(pv_tt)
                for ft in range(EC):
                    nc.tensor.matmul(ops[ft][:],
                                     vt[pv_tt][:, ft * 128:(ft + 1) * 128],
                                     sc_prev[:],
                                     start=(pv_tt == 0), stop=(pv_tt == ST - 1))
        for ft in range(EC):
            o = sbt.tile([128, 512], F32R, name="outT", tag="outT", bufs=8)
            nc.vector.tensor_copy(o[:, 0:256], ops[ft][:, 0:256])
            nc.scalar.copy(o[:, 256:512], ops[ft][:, 256:512])
            s['outT'][ft, sbk] = o

    def emit_wo(h, sbk):
        s = st_h[h]
        wo = s['w']['wo']
        for st in range(sbk * 4, sbk * 4 + 4):
            j = st % 4
            wops = sm_ps()
            for ft in range(EC):
                nc.tensor.matmul(wops[:],
                                 s['outT'][ft, sbk][:, j * 128:(j + 1) * 128],
                                 wo[:, ft * E:(ft + 1) * E],
                                 start=(ft == 0), stop=(ft == EC - 1))
            asl = acc[:, st * E:(st + 1) * E]
            qb, qj = divmod(st, 4)
            eqcol = s['eq2q'][qb][:, qj:qj + 1]
            if h == 0:
                nc.vector.tensor_scalar(asl, wops[:], eqcol, None, OP.mult)
            else:
                nc.vector.scalar_tensor_tensor(asl, wops[:], eqcol,
                                               asl, OP.mult, OP.add)
        if h == N_HEADS_BUILD - 1:
            half, sth = divmod(sbk * 4, 8)
            tgt = bounce_view[half][:, sth:sth + 4, :]
            if sbk < SB - 1:
                nc.sync.dma_start(
                    tgt,
                    acc[:, sbk * 4 * E:(sbk + 1) * 4 * E]
                    .rearrange("p (t e) -> p t e", e=E))
            else:
                nc.sync.dma_start(
                    tgt[:, 0:2, :],
                    acc[:, sbk * 4 * E:(sbk * 4 + 2) * E]
                    .rearrange("p (t e) -> p t e", e=E))
                for ei in range(2, 4):
                    st0 = sbk * 4 + ei
                    nc.sync.dma_start(
                        tgt[:, ei:ei + 1, :],
                        acc[:, st0 * E:(st0 + 1) * E]
                        .rearrange("p (t e) -> p t e", e=E))

    # ============ emission schedule ============
    if N_HEADS_BUILD == 0:
        nc.any.memset(acc[:], 0.0)
    else:
        # LN interleaved with head-0 K projections: main(0,0) needs K/k2 of
        # all four s-blocks, so those quantization chains are the warmup
        # critical path (K squares on ACT there, bias inline).
        for sbk in range(SB):
            emit_ln(sbk)
            if sbk == 0:
                new_head_state(0)
            emit_proj(0, sbk, 'k')
        emit_projA(0, 0, 'q')
        emit_projA(0, 1, 'q')
        for sbk in range(SB):
            emit_v(0, sbk)
        emit_bias(0, 0, 'q')
        emit_bias(0, 1, 'q')

    for h in range(N_HEADS_BUILD):
        nxt = h + 1
        if nxt < N_HEADS_BUILD:
            new_head_state(nxt)
        for sbk in range(SB):
            if sbk < SB - 1:
                wn = (h, sbk + 1)
            elif nxt < N_HEADS_BUILD:
                wn = (nxt, 0)
            else:
                wn = None
            emit_main(h, sbk, warm_next=wn)
            if h == 0:
                # finish head 0's own pieces
                if sbk == 0:
                    emit_projA(0, 2, 'q')
                    emit_projA(0, 3, 'q')
                elif sbk == 1:
                    emit_bias(0, 2, 'q')
                elif sbk == 2:
                    emit_bias(0, 3, 'q')
            if (FP8_V or FP8_PROJ) and h == 0:
                emit_xn8(sbk)
            if nxt < N_HEADS_BUILD:
                if sbk == 0:
                    if h > 0:
                        emit_bias(h, 2, 'q')
                    emit_projA(nxt, 0, 'k')
                    emit_projA(nxt, 1, 'k')
                elif sbk == 1:
                    if h > 0:
                        emit_bias(h, 3, 'q')
                    emit_projA(nxt, 2, 'k')
                    emit_projA(nxt, 3, 'k')
                    emit_bias(nxt, 0, 'k')
                    emit_bias(nxt, 1, 'k')
                elif sbk == 2:
                    emit_bias(nxt, 2, 'k')
                    emit_bias(nxt, 3, 'k')
                    emit_projA(nxt, 0, 'q')
                    emit_projA(nxt, 1, 'q')
                    emit_v(nxt, 0)
                    emit_v(nxt, 1)
                else:
                    emit_bias(nxt, 0, 'q')
                    emit_bias(nxt, 1, 'q')
                    emit_projA(nxt, 2, 'q')
                    emit_projA(nxt, 3, 'q')
                    emit_v(nxt, 2)
                    emit_v(nxt, 3)
            else:
                if sbk == 0:
                    emit_bias(h, 2, 'q')
                elif sbk == 1:
                    emit_bias(h, 3, 'q')
            emit_wo(h, sbk)
        if h > 0:
            st_h.pop(h - 1, None)

    if N_HEADS_BUILD == 0:
        for half in range(2):
            nc.sync.dma_start(
                bounce_view[half][:, :, :],
                acc[:, half * 8 * E:(half + 1) * 8 * E]
                .rearrange("p (t e) -> p t e", e=E))

    # ==== AllReduce over batch pair + store (quarters; last one split) ====
    bos = [dram.tile([S // 2, E], F32, name=f"bounce_out{i}",
                     tag=f"bout{i}", bufs=1) for i in range(2)]
    chunks = [(0, 0, 512), (0, 512, 512), (1, 0, 512), (1, 512, 256),
              (1, 768, 128), (1, 896, 128)]
    for half, r0, rn in chunks:
        rsl = slice(r0, r0 + rn)
        o0 = half * (S // 2) + r0
        osl = out_ext[o0:o0 + rn, :]
        if NO_COLL:
            nc.sync.dma_start(osl, bounce_in[half][rsl, :])
        else:
            nc.gpsimd.collective_compute(
                "AllReduce", OP.add,
                replica_groups=[[0, 1], [2, 3], [4, 5], [6, 7]],
                ins=[bounce_in[half][rsl, :].opt()],
                outs=[bos[half][rsl, :].opt()],
            )
            nc.sync.dma_start(osl, bos[half][rsl, :])


# ================= host side =================

def prep_inputs(x, ln_scale, W_q, W_k, W_v, W_o, gamma):
    """Build per-core input maps."""
    x = np.asarray(x, np.float32)
    ln_scale = np.asarray(ln_scale, np.float32)
    W_q = np.asarray(W_q, np.float32)
    W_k = np.asarray(W_k, np.float32)
    W_v = np.asarray(W_v, np.float32)
    W_o = np.asarray(W_o, np.float32)
    gamma = np.asarray(gamma, np.float32).reshape(H)

    in_maps = []
    for c in range(N_CORES):
        b = c // 2
        h0 = HL * (c % 2)
        hs = list(range(h0, h0 + HL))
        g = gamma[hs]
        s2g = np.sqrt(2.0 * g).astype(np.float32)
        wq = (W_q[hs] * ln_scale[None, :, None] * s2g[:, None, None])
        wk = (W_k[hs] * ln_scale[None, :, None] * s2g[:, None, None])
        wv = (W_v[hs] * ln_scale[None, :, None])
        def _lay(w):   # [HL, E_in(=EC*128), E] -> [HL, 128, EC*E]
            return np.ascontiguousarray(
                w.reshape(HL, EC, 128, E).transpose(0, 2, 1, 3).reshape(HL, 128, EC * E))
        import ml_dtypes
        FP8NP = ml_dtypes.float8_e4m3

        def _split8(w):   # [HL, 128, EC*E] -> fp8 hi/lo [HL, 128, EC, E]
            # x64 lifts the ~0.03-rms folded weights out of fp8e4's
            # subnormal floor; the PSUM drains descale by 1/64.
            w4 = w.reshape(HL, 128, EC, E) * np.float32(64.0)
            hi = w4.astype(FP8NP)
            lo = (w4 - hi.astype(np.float32)).astype(FP8NP)
            return np.ascontiguousarray(hi), np.ascontiguousarray(lo)

        wq = _lay(wq)
        wk = _lay(wk)
        wv = _lay(wv)
        wo = _lay(np.stack([W_o[:, 256 * h:256 * (h + 1)].T.copy() for h in hs]))
        wq8h, wq8l = _split8(wq)
        wk8h, wk8l = _split8(wk)
        wv8h, wv8l = _split8(wv)
        in_maps.append({
            "x": np.ascontiguousarray(x[b]).astype(np.float16),
            "wq": np.ascontiguousarray(wq),
            "wk": np.ascontiguousarray(wk),
            "wv": np.ascontiguousarray(wv),
            "wo": np.ascontiguousarray(wo),
            "wq8h": wq8h, "wq8l": wq8l,
            "wk8h": wk8h, "wk8l": wk8l,
            "wv8h": wv8h, "wv8l": wv8l,
        })
    return in_maps


def assemble_output(results):
    out = np.empty((B, S, E), np.float32)
    for b in range(B):
        out[b] = results[2 * b]["out"]
    return out


_NC_CACHE = {}


def _get_nc():
    if 'nc' not in _NC_CACHE:
        _NC_CACHE['nc'] = build_kernel(R=1, debug=False)
    return _NC_CACHE['nc']


def kernel(x, e=None, p=None, ln_scale=None, W_q=None, W_k=None, W_v=None,
           W_o=None, gamma=None, **_unused):
    """Full-input entry point. e and p are unused by the reference network
    (use_ppe=False config); they are accepted and ignored."""
    in_maps = prep_inputs(x, ln_scale, W_q, W_k, W_v, W_o, gamma)
    nc = _get_nc()
    res = run_bass_kernel_spmd(nc, in_maps, core_ids=list(range(N_CORES)))
    return assemble_output(res.results)
